# revision 2
# baseline (speedup 1.0000x reference)
"""CTC total-loss kernel for nn_CTCLoss_77902116814953 on 8 trn2 NeuronCores.

Strategy (data-parallel over batch + alpha/beta time-split): 64 examples x
{forward, backward} = 128 half-DPs; cores 0-3 run forward halves (16
examples each), cores 4-7 run backward halves of the same examples.
Backward runs on a reversed lattice carrying M = p*beta as its state, so
the instruction stream is identical to forward (SPMD requirement); per-core
behavior differs only through input tensors (one-hots, masks, shift
matrices, acts slab ordering).

Per core: 16 units x lattice S=513 split into 8 chunks x 65 owned states,
window 81 (+1 pad col) with a 16-state halo, SBUF partition p = 8u + c.
Probability-space DP in bf16 (4 vector-engine tensor_tensor ops per time
step), exp-prescale folded into the one-hot gather, halo refresh via a
static PE shift-matmul every 8 steps (evacuated on the scalar engine, off
the DVE critical path), renorm to 2^30 every 64 steps with exact log-scale
bookkeeping (accepted-loss analysis: bf16 + fp32-range truncation effects
are ~1e-4 relative on the final loss vs a 2e-2 budget).

Feed pipeline (hidden under the DVE DP): DMA acts slabs -> PE transpose ->
ACT exp -> e_vt bf16 -> fp8 one-hot gather matmuls (contract over vocab) ->
ACT evac bf16 -> contiguous SBUF->SBUF restage DMAs into per-(u,c)-partition
slab-major p tiles. Z_t = sum_v exp(acts) via selector matmuls, shipped out
for the host softmax correction. Host does the trivial stitch:
P = sum_s alpha_511[s] * beta_511[s] per example,
loss = -(log P - renorm scales - 1024*log C - sum_t log Z_t).

Falls back to a jax pmap implementation, then pure numpy, if the Bass
toolchain is unavailable or input shapes deviate from the spec.
"""

import numpy as np

NEG_INF = np.float32(-1e30)

T, B, V, L = 1024, 64, 512, 256
S = 2 * L + 1
NU = 16          # units per core
NCH = 8          # lattice chunks
OWN, HALO = 65, 16
W = OWN + HALO   # 81
SLAB = 82        # padded slab stride (bf16 -> 164B, 4B-aligned)
R = 8            # halo refresh cadence
RENORM = 64      # renorm cadence (applied while k <= 448)
NSTEP = 512      # update ops per side; the 513th slab is a ones tile
NQ = 4
QS = 128
C_PRE = np.float32(0.25)
TARGET = np.float32(2.0 ** 30)
N_REN = 7

try:
    import ml_dtypes
    import concourse.bass as bass
    import concourse.tile as tile
    from concourse import mybir
    from concourse.bass_utils import run_bass_kernel_spmd

    _HAVE_BASS = True
    F32 = mybir.dt.float32
    BF16 = mybir.dt.bfloat16
    F8 = mybir.dt.float8e4
    AX = mybir.AluOpType
    bfloat16 = ml_dtypes.bfloat16
except Exception:  # pragma: no cover - toolchain not present
    _HAVE_BASS = False


# ======================================================================
# Walrus workaround: split multi-wait instructions
# ======================================================================
def _split_multiwaits(nc):
    """This walrus build rejects instructions with >1 sync wait. Hoist the
    extra waits onto same-engine InstNoOp carriers inserted just before the
    offending instruction (engines run their stream in order, so blocking on
    the NoOps first is equivalent)."""
    n_split = 0
    for bass_bb in nc.bb_map.values():
        bb = bass_bb.bb
        insts = list(bb.instructions)
        out = []
        changed = False
        for inst in insts:
            si = getattr(inst, "sync_info", None)
            waits = list(si.on_wait) if si is not None else []
            if len(waits) > 1:
                changed = True
                n_split += 1
                for w in waits[:-1]:
                    nop = mybir.InstNoOp(
                        name=nc.get_next_instruction_name(), ins=[], outs=[]
                    )
                    nop.engine = inst.engine
                    nop.sync_info = mybir.SyncInfo(on_update=[], on_wait=[w])
                    nc.register_instruction(nop)
                    out.append(nop)
                si.on_wait = waits[-1:]
            out.append(inst)
        if changed:
            bb.instructions = out
    return n_split


# ======================================================================
# Bass kernel builder
# ======================================================================
def _build_nc():
    nc = bass.Bass()
    acts_sh = nc.declare_dram_parameter("acts_sh", [NU, 4, QS, V], F32,
                                        isOutput=False)
    ohh = nc.declare_dram_parameter("ohh", [128, NU, 4, NCH, SLAB], F8,
                                    isOutput=False)
    skipm_in = nc.declare_dram_parameter("skipm", [128, 80], BF16, isOutput=False)
    initm_in = nc.declare_dram_parameter("initm", [128, SLAB], BF16,
                                         isOutput=False)
    shiftm_in = nc.declare_dram_parameter("shiftm", [128, 128], BF16,
                                          isOutput=False)
    ublk_in = nc.declare_dram_parameter("ublk", [128, 128], BF16, isOutput=False)
    ident_in = nc.declare_dram_parameter("ident", [128, 128], F32, isOutput=False)
    ucols_in = nc.declare_dram_parameter("ucols", [128, NU * NU], BF16,
                                         isOutput=False)
    out_a = nc.declare_dram_parameter("out_a", [128, SLAB], F32, isOutput=True)
    out_b = nc.declare_dram_parameter("out_b", [128, SLAB], F32, isOutput=True)
    out_scale = nc.declare_dram_parameter("out_scale", [128, N_REN], F32,
                                          isOutput=True)
    out_z = nc.declare_dram_parameter("out_z", [NU, NSTEP], F32, isOutput=True)

    with tile.TileContext(nc) as tc:
        _emit(tc, nc, acts_sh, ohh, skipm_in, initm_in, shiftm_in, ublk_in,
              ident_in, ucols_in, out_a, out_b, out_scale, out_z)
    _split_multiwaits(nc)
    return nc


def _emit(tc, nc, acts_sh, ohh, skipm_in, initm_in, shiftm_in, ublk_in,
          ident_in, ucols_in, out_a, out_b, out_scale, out_z):
    from contextlib import ExitStack

    ctx = ExitStack()
    with ctx:
        const = ctx.enter_context(tc.tile_pool(name="const", bufs=1))
        pq_pool = ctx.enter_context(tc.tile_pool(name="pq", bufs=1))
        state = ctx.enter_context(tc.tile_pool(name="state", bufs=1))
        tmp = ctx.enter_context(tc.tile_pool(name="tmp", bufs=2))
        actsp = ctx.enter_context(tc.tile_pool(name="actsp", bufs=2))
        evtp = ctx.enter_context(tc.tile_pool(name="evt", bufs=2))
        stgp = ctx.enter_context(tc.tile_pool(name="stg", bufs=3))
        ps_tr = ctx.enter_context(tc.tile_pool(name="ps_tr", bufs=2, space="PSUM"))
        ps_g = ctx.enter_context(tc.tile_pool(name="ps_g", bufs=2, space="PSUM"))
        ps_z = ctx.enter_context(tc.tile_pool(name="ps_z", bufs=1, space="PSUM"))
        ps_sm = ctx.enter_context(tc.tile_pool(name="ps_sm", bufs=1, space="PSUM"))

        skipm = const.tile([128, 80], BF16, tag="skipm")
        nc.sync.dma_start(skipm[:], skipm_in[:])
        initm = const.tile([128, SLAB], BF16, tag="initm")
        nc.sync.dma_start(initm[:], initm_in[:])
        shiftm = const.tile([128, 128], BF16, tag="shiftm")
        nc.sync.dma_start(shiftm[:], shiftm_in[:])
        ublk = const.tile([128, 128], BF16, tag="ublk")
        nc.sync.dma_start(ublk[:], ublk_in[:])
        ident = const.tile([128, 128], F32, tag="ident")
        nc.sync.dma_start(ident[:], ident_in[:])
        ucols = const.tile([128, NU * NU], BF16, tag="ucols")
        nc.sync.dma_start(ucols[:], ucols_in[:])
        ones_slab = const.tile([128, SLAB], BF16, tag="ones_slab")
        nc.vector.memset(ones_slab[:], 1.0)
        zsb = const.tile([NU, NSTEP], F32, tag="zsb")
        scalebuf = const.tile([128, N_REN], F32, tag="scalebuf")

        # p slab storage: 8 half-quarter tiles (64 slabs each) for
        # fine-grained feed->DP dependencies
        pq = []
        for j in range(2 * NQ):
            t_ = pq_pool.tile([128, 64 * SLAB], BF16, name=f"pqh{j}",
                              tag=f"pq{j}")
            pq.append(t_)

        A = [
            state.tile([128, SLAB], BF16, name=f"Abuf{i}", tag=f"A{i}")
            for i in range(2)
        ]

        zps = ps_z.tile([NU, NSTEP], F32, tag="zps")

        # resident one-hots, fp8 (values 0 / C_PRE exactly representable)
        oh_res = const.tile([128, NU * 4 * NCH * SLAB], F8, tag="oh_res")
        nc.sync.dma_start(oh_res[:], ohh.rearrange("v u a c s -> v (u a c s)"))

        def feed_quarter(j):
            for u in range(NU):
                oh_t = oh_res[:, u * 4 * NCH * SLAB:(u + 1) * 4 * NCH * SLAB]
                at = actsp.tile([128, V], F32, tag="acts")
                nc.sync.dma_start(at[:], acts_sh[u, j])
                evt = evtp.tile([128, 4 * QS], BF16, tag="evt")
                for vc in range(4):
                    trp = ps_tr.tile([128, 128], F32, tag="trp")
                    nc.tensor.transpose(trp[:], at[:, vc * 128:(vc + 1) * 128],
                                        ident[:])
                    nc.scalar.activation(evt[:, vc * QS:(vc + 1) * QS], trp[:],
                                         mybir.ActivationFunctionType.Exp)
                # Z rows: the ucols column-block selects output row u; all 64
                # matmuls of a quarter form one accumulation group.
                for vc in range(4):
                    nc.tensor.matmul(
                        zps[:, j * QS:(j + 1) * QS],
                        lhsT=ucols[:, u * NU:(u + 1) * NU],
                        rhs=evt[:, vc * QS:(vc + 1) * QS],
                        start=(u == 0 and vc == 0),
                        stop=(u == NU - 1 and vc == 3),
                        skip_group_check=True,
                    )
                for g in range(2):
                    gp = ps_g.tile([128, 4 * SLAB], F32, tag="gp")
                    for vc in range(4):
                        nc.tensor.matmul(
                            gp[:],
                            lhsT=evt[:, vc * QS:(vc + 1) * QS],
                            rhs=oh_t[:, (vc * NCH + 4 * g) * SLAB:
                                    (vc * NCH + 4 * g + 4) * SLAB],
                            start=(vc == 0), stop=(vc == 3),
                            skip_group_check=True,
                        )
                    stg = stgp.tile([128, 4 * SLAB], BF16, tag="stg")
                    nc.scalar.copy(stg[:], gp[:])
                    for ci in range(4):
                        c = 4 * g + ci
                        p_row = 8 * u + c
                        # contiguous full-SLAB copies (pad col carries psum's
                        # zero pad); one DMA per (u, c, half), issue rings
                        # alternated between SP and ACT HWDGE
                        nc.sync.dma_start(
                            pq[2 * j][p_row:p_row + 1, :],
                            stg[0:64, ci * SLAB:(ci + 1) * SLAB])
                        nc.scalar.dma_start(
                            pq[2 * j + 1][p_row:p_row + 1, :],
                            stg[64:128, ci * SLAB:(ci + 1) * SLAB])

        def dp_op(k):
            src = A[(k + 1) % 2]
            dst = A[k % 2]
            if k < NSTEP:
                hj, kl = divmod(k, 64)
                P = pq[hj][:, kl * SLAB:(kl + 1) * SLAB]
            else:
                P = ones_slab[:]
            t3 = tmp.tile([128, 80], BF16, tag="t3")
            nc.vector.tensor_tensor(t3[:], skipm[:], src[:, 0:80], AX.mult)
            t1 = tmp.tile([128, 80], BF16, tag="t1")
            nc.vector.tensor_tensor(t1[:], src[:, 1:81], src[:, 2:82], AX.add)
            t2 = tmp.tile([128, 80], BF16, tag="t2")
            nc.vector.tensor_tensor(t2[:], t1[:], t3[:], AX.add)
            nc.vector.tensor_tensor(dst[:, 2:82], t2[:], P[:, 2:82], AX.mult)

        def refresh(k):
            cur = A[k % 2]
            rp = ps_sm.tile([128, HALO], F32, tag="rp")
            nc.tensor.matmul(rp[:], lhsT=shiftm[:], rhs=cur[:, OWN:OWN + HALO],
                             start=True, stop=True, skip_group_check=True)
            nc.scalar.copy(cur[:, 0:HALO], rp[:])

        def renorm(k, ridx):
            cur = A[k % 2]
            mx = tmp.tile([128, 1], BF16, tag="mx")
            nc.vector.tensor_reduce(mx[:], cur[:, 0:81],
                                    axis=mybir.AxisListType.X, op=AX.max)
            np_ = ps_sm.tile([128, 1], F32, tag="np")
            nc.tensor.matmul(np_[:], lhsT=ublk[:], rhs=mx[:],
                             start=True, stop=True, skip_group_check=True)
            rv = tmp.tile([128, 1], F32, tag="rv")
            nc.vector.reciprocal(rv[:], np_[:])
            nc.vector.tensor_scalar(
                out=cur[:, 0:81], in0=cur[:, 0:81],
                scalar1=rv[:], scalar2=float(TARGET),
                op0=AX.mult, op1=AX.mult,
            )
            nc.vector.tensor_copy(scalebuf[:, ridx:ridx + 1], rv[:])

        nc.vector.memset(A[0][:], 0.0)
        nc.vector.memset(A[1][:], 0.0)
        feed_quarter(0)
        nc.vector.tensor_tensor(A[0][:, 0:SLAB], pq[0][:, 0:SLAB], initm[:],
                                AX.mult)
        ridx = 0
        for k in range(1, NSTEP + 1):
            if k == 1:
                for jq in range(1, NQ):
                    feed_quarter(jq)
            dp_op(k)
            if k % R == 0:
                refresh(k)
            if k % RENORM == 0 and k <= 448:
                renorm(k, ridx)
                ridx += 1

        fin_a = tmp.tile([128, SLAB], F32, tag="fin")
        nc.scalar.copy(fin_a[:], A[1][:])      # state after op 511
        nc.sync.dma_start(out_a[:], fin_a[:])
        fin_b = tmp.tile([128, SLAB], F32, tag="fin")
        nc.scalar.copy(fin_b[:], A[0][:])      # state after op 512 (ones)
        nc.sync.dma_start(out_b[:], fin_b[:])
        nc.sync.dma_start(out_scale[:], scalebuf[:])
        nc.scalar.copy(zsb[:], zps[:])
        nc.sync.dma_start(out_z[:], zsb[:])


# ======================================================================
# Host-side prep and stitch
# ======================================================================
def _host_prep(acts, labels, act_lens, label_lens):
    acts = np.asarray(acts, np.float32)
    labels = np.asarray(labels).astype(np.int64)
    label_lens = np.asarray(label_lens).astype(np.int64)

    labels2d = labels.reshape(B, L)
    j = np.arange(L)
    padded = np.where(j[None, :] < label_lens[:, None], labels2d, 0)
    ext = np.zeros((B, S), np.int64)
    ext[:, 1::2] = padded
    ext_m2 = np.zeros_like(ext)
    ext_m2[:, 2:] = ext[:, :-2]
    skip = (ext != 0) & (ext != ext_m2)
    skip[:, :2] = False

    s_of = np.arange(NCH)[:, None] * OWN - HALO + np.arange(W)[None, :]
    SPAD = NCH * OWN

    in_maps = []
    stitch = {"label_lens": label_lens}
    for core in range(8):
        is_fwd = core < 4
        exs = np.arange(16 * (core % 4), 16 * (core % 4) + 16)
        a_sh = np.empty((NU, 4, QS, V), np.float32)
        for ui, b in enumerate(exs):
            if is_fwd:
                a_sh[ui] = acts[0:NSTEP, b, :].reshape(4, QS, V)
            else:
                a_sh[ui] = acts[::-1][0:NSTEP, b, :].reshape(4, QS, V)
        if is_fwd:
            s_map = np.broadcast_to(s_of, (NU, NCH, W)).copy()
        else:
            s_map = SPAD - 1 - np.broadcast_to(s_of, (NU, NCH, W)).copy()
        validm = (s_map >= 0) & (s_map < S)
        s_cl = np.clip(s_map, 0, S - 1)
        extw = np.where(validm, ext[exs[:, None, None], s_cl], 0)

        oh = np.zeros((128, NU, 4, NCH, SLAB), np.float32)
        uu, cc, jj = np.nonzero(validm)
        lab = extw[uu, cc, jj]
        oh[lab % 128, uu, lab // 128, cc, jj] = C_PRE
        oh = oh.astype(ml_dtypes.float8_e4m3)

        if is_fwd:
            sk_state = s_map
        else:
            sk_state = s_map + 2
        skv = (sk_state >= 0) & (sk_state < S) & validm
        skw = np.where(skv, skip[exs[:, None, None], np.clip(sk_state, 0, S - 1)],
                       False)
        skipm = np.zeros((128, 80), np.float32)
        for ui in range(NU):
            for c in range(NCH):
                skipm[8 * ui + c, 0:79] = skw[ui, c, 2:81]
        skipm = skipm.astype(bfloat16)

        initm = np.zeros((128, SLAB), np.float32)
        for ui, b in enumerate(exs):
            if is_fwd:
                tgt = (s_map[ui] == 0) | (s_map[ui] == 1)
            else:
                e0, e1 = 2 * label_lens[b], 2 * label_lens[b] - 1
                tgt = (s_map[ui] == e0) | (s_map[ui] == e1)
            for c in range(NCH):
                initm[8 * ui + c, 0:W] = (tgt[c] & validm[ui, c]).astype(np.float32)
        initm = initm.astype(bfloat16)

        shiftm = np.zeros((128, 128), np.float32)
        for ui in range(NU):
            for c in range(1, NCH):
                shiftm[8 * ui + c - 1, 8 * ui + c] = 1.0
        shiftm = shiftm.astype(bfloat16)

        ublk = np.zeros((128, 128), np.float32)
        for ui in range(NU):
            ublk[8 * ui:8 * ui + 8, 8 * ui:8 * ui + 8] = 1.0
        ublk = ublk.astype(bfloat16)

        ident = np.eye(128, dtype=np.float32)

        ucols = np.zeros((128, NU * NU), np.float32)
        for u in range(NU):
            ucols[:, u * NU + u] = 1.0
        ucols = ucols.astype(bfloat16)

        in_maps.append({
            "acts_sh": a_sh,
            "ohh": oh,
            "skipm": skipm,
            "initm": initm,
            "shiftm": shiftm,
            "ublk": ublk,
            "ident": ident,
            "ucols": ucols,
        })
        if core == 0:
            stitch["s_map_f"] = s_map
        if core == 4:
            stitch["s_map_b"] = s_map
    return in_maps, stitch


def _host_stitch(results, stitch):
    label_lens = stitch["label_lens"]
    total = 0.0
    logC = np.log(np.float64(C_PRE))
    logT = np.log(np.float64(TARGET))
    s_map_f, s_map_b = stitch["s_map_f"], stitch["s_map_b"]
    for core in range(4):
        res_f, res_b = results[core], results[core + 4]
        af = np.asarray(res_f["out_a"], np.float32)   # state after op 511
        bb = np.asarray(res_b["out_b"], np.float32)   # state after op 512
        sc_f_all = np.asarray(res_f["out_scale"], np.float32)
        sc_b_all = np.asarray(res_b["out_scale"], np.float32)
        z_f_all = np.asarray(res_f["out_z"], np.float32)
        z_b_all = np.asarray(res_b["out_z"], np.float32)
        for ui in range(NU):
            alpha = np.zeros(S, np.float64)
            beta = np.zeros(S, np.float64)
            for c in range(NCH):
                row_f = af[8 * ui + c]
                row_b = bb[8 * ui + c]
                for jw in range(HALO, W):
                    s = s_map_f[ui, c, jw]
                    if 0 <= s < S:
                        alpha[s] = row_f[jw]
                    sb = s_map_b[ui, c, jw]
                    if 0 <= sb < S:
                        beta[sb] = row_b[jw]
            Phat = float((alpha * beta).sum())
            ls = (np.log(np.float64(sc_f_all[8 * ui])).sum() + N_REN * logT
                  + np.log(np.float64(sc_b_all[8 * ui])).sum() + N_REN * logT)
            logZ = (np.log(z_f_all[ui].astype(np.float64)).sum()
                    + np.log(z_b_all[ui].astype(np.float64)).sum())
            logP = np.log(Phat) - ls - 1024 * logC - logZ
            total += -logP
    return np.asarray([total], np.float32)


_NC_CACHE = None


def _bass_kernel(acts, labels, act_lens, label_lens):
    global _NC_CACHE
    if _NC_CACHE is None:
        _NC_CACHE = _build_nc()
    in_maps, stitch = _host_prep(acts, labels, act_lens, label_lens)
    res = run_bass_kernel_spmd(_NC_CACHE, in_maps, list(range(8)))
    return _host_stitch(res.results, stitch)


# ======================================================================
# jax pmap fallback
# ======================================================================
def _try_pmap_path(acts, labels, act_lens, label_lens):
    try:
        import jax
        import jax.numpy as jnp

        devs = jax.devices()
        n = 8
        if len(devs) < n:
            return None
        T_, B_, V_ = acts.shape
        L_ = labels.shape[0] // B_
        if B_ % n != 0:
            return None
        bs = B_ // n
        labels2d = np.asarray(labels).reshape(B_, L_)

        def shard_loss(acts_s, labels_s, act_lens_s, label_lens_s):
            S_ = 2 * L_ + 1
            logp = jax.nn.log_softmax(acts_s, axis=-1)
            jj = jnp.arange(L_, dtype=jnp.int32)
            padded = jnp.where(jj[None, :] < label_lens_s[:, None], labels_s, 0)
            ext = jnp.zeros((bs, S_), dtype=jnp.int32).at[:, 1::2].set(padded)
            ext_m2 = jnp.pad(ext, ((0, 0), (2, 0)))[:, :S_]
            allow_skip = (ext != 0) & (ext != ext_m2)
            allow_skip = allow_skip.at[:, :2].set(False)
            bi = jnp.arange(bs)[:, None]
            lp_ext = logp[:, bi, ext]
            alpha0 = jnp.full((bs, S_), NEG_INF, dtype=jnp.float32)
            alpha0 = alpha0.at[:, 0].set(lp_ext[0, :, 0]).at[:, 1].set(
                lp_ext[0, :, 1])
            neg = jnp.full((bs, 1), NEG_INF, dtype=jnp.float32)

            def step(alpha, inp):
                lp_t, t = inp
                a2 = jnp.concatenate([neg, alpha[:, :-1]], axis=1)
                a3 = jnp.concatenate([neg, neg, alpha[:, :-2]], axis=1)
                a3 = jnp.where(allow_skip, a3, NEG_INF)
                new = lp_t + jnp.logaddexp(jnp.logaddexp(alpha, a2), a3)
                alpha = jnp.where((t < act_lens_s)[:, None], new, alpha)
                return alpha, None

            alpha, _ = jax.lax.scan(step, alpha0, (lp_ext[1:], jnp.arange(1, T_)))
            s_end = (2 * label_lens_s).astype(jnp.int32)
            a_end = jnp.take_along_axis(alpha, s_end[:, None], axis=1)[:, 0]
            a_end2 = jnp.take_along_axis(
                alpha, jnp.maximum(s_end - 1, 0)[:, None], axis=1)[:, 0]
            return -jnp.sum(jnp.logaddexp(a_end, a_end2))

        pl = jax.pmap(shard_loss, devices=devs[:n])
        acts_sh = np.asarray(acts, np.float32).reshape(
            T_, n, bs, V_).transpose(1, 0, 2, 3)
        labels_sh = labels2d.reshape(n, bs, L_)
        al_sh = np.asarray(act_lens, np.int32).reshape(n, bs)
        ll_sh = np.asarray(label_lens, np.int32).reshape(n, bs)
        per = np.asarray(pl(acts_sh, labels_sh, al_sh, ll_sh))
        if not np.all(np.isfinite(per)):
            return None
        return np.asarray([per.sum()], dtype=np.float32)
    except Exception:
        return None


# ======================================================================
# numpy fallback (reference-equivalent)
# ======================================================================
def _ctc_total_loss_np(acts, labels, act_lens, label_lens):
    acts = np.asarray(acts, dtype=np.float32)
    labels = np.asarray(labels)
    act_lens = np.asarray(act_lens).astype(np.int64)
    label_lens = np.asarray(label_lens).astype(np.int64)

    T_, B_, V_ = acts.shape
    L_ = labels.shape[0] // B_
    S_ = 2 * L_ + 1

    m = acts.max(axis=-1, keepdims=True)
    shifted = acts - m
    logp = shifted - np.log(np.exp(shifted).sum(axis=-1, keepdims=True))

    offsets = np.concatenate([np.zeros(1, np.int64), np.cumsum(label_lens)])[:-1]
    j = np.arange(L_, dtype=np.int64)
    gidx = np.clip(offsets[:, None] + j[None, :], 0, labels.shape[0] - 1)
    padded = np.where(j[None, :] < label_lens[:, None], labels[gidx], 0)

    ext = np.zeros((B_, S_), dtype=np.int64)
    ext[:, 1::2] = padded
    ext_m2 = np.zeros_like(ext)
    ext_m2[:, 2:] = ext[:, :-2]
    allow_skip = (ext != 0) & (ext != ext_m2)
    allow_skip[:, :2] = False

    bidx = np.arange(B_)[:, None]
    lp_ext = logp[:, bidx, ext]

    alpha = np.full((B_, S_), NEG_INF, dtype=np.float32)
    alpha[:, 0] = lp_ext[0, :, 0]
    alpha[:, 1] = lp_ext[0, :, 1]

    neg1 = np.full((B_, 1), NEG_INF, dtype=np.float32)
    neg2 = np.full((B_, 2), NEG_INF, dtype=np.float32)
    for t in range(1, T_):
        a2 = np.concatenate([neg1, alpha[:, :-1]], axis=1)
        a3 = np.concatenate([neg2, alpha[:, :-2]], axis=1)
        a3 = np.where(allow_skip, a3, NEG_INF)
        new = lp_ext[t] + np.logaddexp(np.logaddexp(alpha, a2), a3)
        frozen = t >= act_lens
        if frozen.any():
            alpha = np.where(frozen[:, None], alpha, new)
        else:
            alpha = new

    s_end = 2 * label_lens
    rows = np.arange(B_)
    a_end = alpha[rows, s_end]
    a_end2 = alpha[rows, np.maximum(s_end - 1, 0)]
    losses = -np.logaddexp(a_end, a_end2)
    return np.asarray([losses.sum()], dtype=np.float32)


# ======================================================================
# entry point
# ======================================================================
def kernel(acts, labels, act_lens, label_lens):
    acts = np.asarray(acts)
    labels = np.asarray(labels)
    act_lens = np.asarray(act_lens)
    label_lens = np.asarray(label_lens)

    spec_ok = (
        acts.shape == (T, B, V)
        and labels.shape == (B * L,)
        and np.all(np.asarray(act_lens) == T)
        and np.all(np.asarray(label_lens) == L)
    )
    if _HAVE_BASS and spec_ok:
        try:
            return _bass_kernel(acts, labels, act_lens, label_lens)
        except Exception:
            pass
    out = _try_pmap_path(acts, labels, act_lens, label_lens)
    if out is not None:
        return out
    return _ctc_total_loss_np(acts, labels, act_lens, label_lens)
